# revision 39
# baseline (speedup 1.0000x reference)
"""TRN2 Bass kernel for DotProductAttention (N=16, Cx=256, Tx=Ty=2048, fp32).

reference:
    scores = einsum('nct,ncy->nty', K, Q) / sqrt(Cx)   # (N, Tx, Ty)
    A = softmax(scores, axis=1)                        # over Tx
    R = einsum('nct,nty->ncy', V, A)                   # (N, Cx, Ty)
    returns (R, A)

Sharding: data-parallel over batch N across 8 cores (2 batches per core).

Per-core plan: jobs = (batch, ty-slab of 512), software-pipelined, with all
PE work as regular (warm) matmuls — no PE-transpose of the score matrix:
  phase A: S[tx,ty] = K^T Q in natural layout on PE (fp32r, contraction Cx
           on partitions), exp(S/16) on ScalarE straight out of PSUM, and
           column sums via ones-vector matmuls accumulating into a [1,SLAB]
           PSUM row (M=1 matmul, free dim 512).
  r-path:  r = 1/sums (VectorE on the [1,SLAB] row), gpsimd partition
           broadcast to [128,SLAB].
  phase B: per pair of tx tiles: fused normalize-copy on VectorE
           (exp_tile * r_bcast -> A tile), DMA A out, accumulate
           R += V^T.T @ A on PE (fp32r).
Phase-A work of job k+1 is emitted interleaved into phase-B rounds of job k
so PE never drains while the r-path of a slab resolves.
All matmul inputs are fp32r (~13-bit mantissa, 1 cycle/row at free dim 512).
"""

import numpy as np

import concourse.bass as bass
import concourse.tile as tile
import concourse.mybir as mybir
from concourse import bacc
from concourse.bass_utils import run_bass_kernel_spmd
from concourse.masks import make_identity

P = 128
CX = 256
T = 2048
NB = 2            # batches per core
SLAB = 512        # ty slab size
N_CORES = 8
SCALE = 1.0 / 16.0  # 1/sqrt(CX)

f32 = mybir.dt.float32
f32r = mybir.dt.float32r

CXO = CX // P             # 2 cx chunks
NJ = T // P               # 16 tx tiles of 128
NJ2 = NJ // 2             # 8 pairs of tx tiles (pipeline rounds)
NSLAB = T // SLAB         # 4 slabs per batch
ONES_LAG = 2              # pairs of lag before the ones-matmul reads exp

_NC_CACHE = None


def build(krep=1, bench_internal_a=False, no_adma=False, phase1_only=False,
          no_rmm=False, bench_no_io=False):
    """krep>1 wraps the whole body in a hardware loop that recomputes the
    same outputs krep times — used only for differential HW timing.
    bench_internal_a keeps A in device DRAM (not transferred back).
    bench_no_io keeps ALL tensors on device (zero-filled once) with a dummy
    4-byte output, so call wall time is dispatch + execute only.
    no_adma / phase1_only / no_rmm are timing-bisection modes (wrong
    outputs)."""
    nc = bacc.Bacc(None, target_bir_lowering=False)

    io_kind = "Internal" if bench_no_io else "ExternalInput"
    K_d = nc.dram_tensor("K", [NB, CX, T], f32r, kind=io_kind)
    V_d = nc.dram_tensor("V", [NB, CX, T], f32r, kind=io_kind)
    Q_d = nc.dram_tensor("Q", [NB, CX, T], f32r, kind=io_kind)
    a_kind = ("Internal" if (bench_internal_a or bench_no_io)
              else "ExternalOutput")
    A_d = nc.dram_tensor("A", [NB, T, T], f32r, kind=a_kind)
    r_kind = "Internal" if bench_no_io else "ExternalOutput"
    R_d = nc.dram_tensor("R", [NB, CX, T], f32, kind=r_kind)
    dummy_d = (nc.dram_tensor("bench_out", [1, 1], f32, kind="ExternalOutput")
               if bench_no_io else None)

    with tile.TileContext(nc) as tc:
        with (
            tc.tile_pool(name="consts", bufs=1) as consts,
            tc.tile_pool(name="kq", bufs=2) as kq,
            tc.tile_pool(name="vv", bufs=1) as vv,
            tc.tile_pool(name="vt", bufs=2) as vtp,
            tc.tile_pool(name="en", bufs=12) as enp,
            tc.tile_pool(name="aj", bufs=4) as ajp,
            tc.tile_pool(name="rout", bufs=4) as routp,
            tc.tile_pool(name="rmisc", bufs=3) as rmisc,
            tc.tile_pool(name="ps", bufs=2, space="PSUM") as psp,
            tc.tile_pool(name="ps_ones", bufs=2, space="PSUM") as ps_ones,
            tc.tile_pool(name="ps_r", bufs=2, space="PSUM") as ps_r,
        ):
            ident = consts.tile([P, P], f32)
            make_identity(nc, ident)
            identr = consts.tile([P, P], f32r)
            nc.vector.tensor_copy(identr[:], ident[:])
            ones_f = consts.tile([P, 1], f32)
            nc.vector.memset(ones_f[:], 1.0)
            ones = consts.tile([P, 1], f32r)
            nc.vector.tensor_copy(ones[:], ones_f[:])

            jobs = [(b, s) for b in range(NB) for s in range(NSLAB)]
            kq_tiles = {}     # b -> (k_sb, q_sb)
            vt_tiles = {}     # b -> vt
            jstate = {}       # job index -> (en_pairs, ones_psum, pending)
            v_pending = {}    # b -> v_sb awaiting VT build

            def emit_kqv_load(b):
                k_sb = kq.tile([P, CXO, T], f32r, tag="K", name=f"k{b}")
                q_sb = kq.tile([P, CXO, T], f32r, tag="Q", name=f"q{b}")
                kq_tiles[b] = (k_sb, q_sb)
                kre = K_d[b].rearrange("(o p) t -> p o t", p=P)
                qre = Q_d[b].rearrange("(o p) t -> p o t", p=P)
                # slab-0 phase A first needs q[0:512] and k progressively
                cuts = [(q_sb, qre, 0, 512), (k_sb, kre, 0, 512),
                        (k_sb, kre, 512, 1024), (k_sb, kre, 1024, 1536),
                        (k_sb, kre, 1536, 2048), (q_sb, qre, 512, 1024),
                        (q_sb, qre, 1024, 1536), (q_sb, qre, 1536, 2048)]
                for dst, src, lo, hi in cuts:
                    nc.sync.dma_start(dst[:, :, lo:hi], src[:, :, lo:hi])
                v_sb = vv.tile([P, CXO, T], f32r, tag="V", name=f"v{b}")
                nc.sync.dma_start(
                    v_sb[:], V_d[b].rearrange("(o p) t -> p o t", p=P))
                return v_sb

            def emit_vt_build(b, v_sb):
                vt = vtp.tile([P, NJ, CX], f32r, tag="VT", name=f"vt{b}")
                vt_tiles[b] = vt
                for j in range(NJ):
                    for c in range(CXO):
                        pvt = psp.tile([P, P], f32r, tag="work", name="pvt")
                        nc.tensor.transpose(
                            pvt[:], v_sb[:, c, j * P:(j + 1) * P], identr[:])
                        nc.vector.tensor_copy(
                            vt[:, j, c * P:(c + 1) * P], pvt[:])

            def _get_state(ji):
                if ji not in jstate:
                    jstate[ji] = ([None] * NJ2,
                                  ps_ones.tile([1, SLAB], f32, tag="ones",
                                               name=f"os{ji}"),
                                  [])
                return jstate[ji]

            def emit_ones_mm(ji, jp):
                en_pairs, osum, _ = jstate[ji]
                en = en_pairs[jp]
                for h in range(2):
                    nc.tensor.matmul(
                        osum[:], ones[:], en[:, h, :],
                        start=(jp == 0 and h == 0),
                        stop=(jp == NJ2 - 1 and h == 1))

            def emit_phaseA_pair(ji, jp):
                """Scores matmuls + exp for one pair of tx tiles; queues the
                ones-matmul to run ONES_LAG pairs later (so the in-order PE
                stream doesn't wait on ScalarE's exp)."""
                b, s = jobs[ji]
                en_pairs, osum, pending = _get_state(ji)
                k_sb, q_sb = kq_tiles[b]
                ty0 = s * SLAB
                en = enp.tile([P, 2, SLAB], f32r, tag="en",
                              name=f"en{ji}_{jp}")
                en_pairs[jp] = en
                pss = psp.tile([P, 2, SLAB], f32, tag="work", name="pss")
                for h in range(2):
                    j = jp * 2 + h
                    for c in range(CXO):
                        nc.tensor.matmul(
                            pss[:, h, :],
                            k_sb[:, c, j * P:(j + 1) * P],
                            q_sb[:, c, ty0:ty0 + SLAB],
                            start=(c == 0), stop=(c == CXO - 1))
                nc.scalar.activation(
                    out=en[:], in_=pss[:],
                    func=mybir.ActivationFunctionType.Exp, scale=SCALE)
                pending.append(jp)
                if len(pending) > ONES_LAG:
                    emit_ones_mm(ji, pending.pop(0))
                if jp == NJ2 - 1:
                    while pending:
                        emit_ones_mm(ji, pending.pop(0))

            def emit_rpath(ji):
                """r = 1/colsums on the [1, SLAB] psum row, broadcast to all
                partitions. No PE involvement."""
                _, osum, _ = jstate[ji]
                rrow = rmisc.tile([1, SLAB], f32, tag="rrow")
                nc.vector.reciprocal(rrow[:], osum[:])
                rbc = rmisc.tile([P, SLAB], f32, tag="rbc")
                nc.gpsimd.partition_broadcast(rbc[:], rrow[:])
                return rbc

            def emit_phaseB_pair(ji, jp, rbc, pr):
                """R accumulation straight from the unnormalized exp tiles
                (R = (V.E).diag(r), normalized at psum evacuation), plus the
                A-output normalize-copy as a VectorE/DMA side stream."""
                b, s = jobs[ji]
                en_pairs, _, _ = jstate[ji]
                vt = vt_tiles[b]
                ty0 = s * SLAB
                en = en_pairs[jp]
                if not no_rmm:
                    for h in range(2):
                        j = jp * 2 + h
                        for c in range(CXO):
                            nc.tensor.matmul(
                                pr[c][:], vt[:, j, c * P:(c + 1) * P],
                                en[:, h, :],
                                start=(j == 0), stop=(j == NJ - 1))
                aj = ajp.tile([P, 2, SLAB], f32r, tag="aj")
                nc.vector.tensor_tensor(
                    aj[:], en[:],
                    rbc[:].unsqueeze(1).to_broadcast((P, 2, SLAB)),
                    mybir.AluOpType.mult)
                if not no_adma:
                    for h in range(2):
                        j = jp * 2 + h
                        nc.sync.dma_start(
                            A_d[b, j * P:(j + 1) * P, ty0:ty0 + SLAB],
                            aj[:, h, :])

            def next_job_emitters(nji):
                """Emitters for job nji's phase A (+ VT build at batch start).
                K/Q/V loads are prefetched separately two jobs ahead."""
                if nji >= len(jobs):
                    return []
                b, s = jobs[nji]
                ems = []
                if b not in kq_tiles:
                    ems.append(lambda b=b: v_pending.__setitem__(
                        b, emit_kqv_load(b)))
                for jp in range(NJ2):
                    ems.append(lambda ji=nji, jp=jp: emit_phaseA_pair(ji, jp))
                if s == 0:
                    ems.append(lambda b=b: emit_vt_build(b, v_pending.pop(b)))
                return ems

            # ---- main pipeline ----
            def _pipeline():
              kq_tiles.clear()
              vt_tiles.clear()
              v_pending.clear()
              jstate.clear()
              for e in next_job_emitters(0):
                e()
              for ji, (b, s) in enumerate(jobs):
                rbc = emit_rpath(ji)
                # prefetch next batch's inputs two jobs ahead
                if ji + 2 < len(jobs):
                    nb = jobs[ji + 2][0]
                    if nb not in kq_tiles:
                        v_pending[nb] = emit_kqv_load(nb)
                if phase1_only:
                    for e in next_job_emitters(ji + 1):
                        e()
                    for c in range(CXO):
                        r_sb = routp.tile([P, SLAB], f32, tag="rsb")
                        nc.vector.memset(r_sb[:], 0.0)
                        nc.sync.dma_start(
                            R_d[b, c * P:(c + 1) * P,
                                s * SLAB:(s + 1) * SLAB], r_sb[:])
                    del jstate[ji]
                    continue
                nxt = next_job_emitters(ji + 1)
                pr = [ps_r.tile([P, SLAB], f32, tag="psr", name=f"psr{ji}_{c}")
                      for c in range(CXO)]
                # lead with next-job phase-A work so PE has rbc-independent
                # instructions while the r-path resolves
                for _ in range(2):
                    if nxt:
                        nxt.pop(0)()
                for jp in range(NJ2):
                    emit_phaseB_pair(ji, jp, rbc, pr)
                    if nxt:
                        nxt.pop(0)()
                while nxt:
                    nxt.pop(0)()
                for c in range(CXO):
                    r_sb = routp.tile([P, SLAB], f32, tag="rsb")
                    if no_rmm:
                        nc.vector.memset(r_sb[:], 0.0)
                    else:
                        # normalize R during psum evacuation: R = (V.E) * r
                        nc.vector.tensor_tensor(
                            r_sb[:], pr[c][:], rbc[:], mybir.AluOpType.mult)
                    nc.sync.dma_start(
                        R_d[b, c * P:(c + 1) * P, s * SLAB:(s + 1) * SLAB],
                        r_sb[:])
                del jstate[ji]

            if bench_no_io:
                # zero-fill the internal input tensors once so the compute
                # sees finite data (exp(0)=1), and write the dummy output
                zsb = consts.tile([P, 2048], f32r, name="zsb")
                nc.vector.memset(zsb[:].bitcast(f32), 0.0)
                for td in (K_d, V_d, Q_d):
                    flat = td[:].rearrange("b c t -> (b c) t")
                    for h in range(4):
                        nc.sync.dma_start(flat[h * P:(h + 1) * P, :], zsb[:])
                dsb = consts.tile([1, 1], f32, name="dsb")
                nc.vector.memset(dsb[:], 0.0)
                nc.sync.dma_start(dummy_d[:], dsb[:])

            if krep > 1:
                with tc.For_i(0, krep, 1):
                    _pipeline()
            else:
                _pipeline()

    nc.compile()
    return nc


def kernel(K, V, Q):
    global _NC_CACHE
    K = np.ascontiguousarray(np.asarray(K, dtype=np.float32))
    V = np.ascontiguousarray(np.asarray(V, dtype=np.float32))
    Q = np.ascontiguousarray(np.asarray(Q, dtype=np.float32))
    N = K.shape[0]
    assert K.shape == (N, CX, T) and V.shape == (N, CX, T) and Q.shape == (N, CX, T)

    if _NC_CACHE is None:
        _NC_CACHE = build()
    nc = _NC_CACHE

    in_maps = [
        {"K": K[c * NB:(c + 1) * NB], "V": V[c * NB:(c + 1) * NB],
         "Q": Q[c * NB:(c + 1) * NB]}
        for c in range(N_CORES)
    ]
    res = run_bass_kernel_spmd(nc, in_maps, core_ids=list(range(N_CORES)))
    A = np.concatenate([r["A"] for r in res.results], axis=0)
    R = np.concatenate([r["R"] for r in res.results], axis=0)
    return (R, A)


# revision 42
# speedup vs baseline: 1.1336x; 1.1336x over previous
"""TRN2 Bass kernel for DotProductAttention (N=16, Cx=256, Tx=Ty=2048, fp32).

reference:
    scores = einsum('nct,ncy->nty', K, Q) / sqrt(Cx)   # (N, Tx, Ty)
    A = softmax(scores, axis=1)                        # over Tx
    R = einsum('nct,nty->ncy', V, A)                   # (N, Cx, Ty)
    returns (R, A)

Sharding: data-parallel over batch N across 8 cores (2 batches per core).

Per-core plan: jobs = (batch, ty-slab of 512), software-pipelined, with all
PE work as regular (warm) matmuls — no PE-transpose of the score matrix:
  phase A: S[tx,ty] = K^T Q in natural layout on PE (fp32r, contraction Cx
           on partitions), exp(S/16) on ScalarE straight out of PSUM, and
           column sums via ones-vector matmuls accumulating into a [1,SLAB]
           PSUM row (M=1 matmul, free dim 512).
  r-path:  r = 1/sums (VectorE on the [1,SLAB] row), gpsimd partition
           broadcast to [128,SLAB].
  phase B: per pair of tx tiles: fused normalize-copy on VectorE
           (exp_tile * r_bcast -> A tile), DMA A out, accumulate
           R += V^T.T @ A on PE (fp32r).
Phase-A work of job k+1 is emitted interleaved into phase-B rounds of job k
so PE never drains while the r-path of a slab resolves.
All matmul inputs are fp32r (~13-bit mantissa, 1 cycle/row at free dim 512).
"""

import numpy as np

import concourse.bass as bass
import concourse.tile as tile
import concourse.mybir as mybir
from concourse import bacc
from concourse.bass_utils import run_bass_kernel_spmd
from concourse.masks import make_identity

P = 128
CX = 256
T = 2048
NB = 2            # batches per core
SLAB = 512        # ty slab size
N_CORES = 8
SCALE = 1.0 / 16.0  # 1/sqrt(CX)

f32 = mybir.dt.float32
f32r = mybir.dt.float32r

CXO = CX // P             # 2 cx chunks
NJ = T // P               # 16 tx tiles of 128
NJ2 = NJ // 2             # 8 pairs of tx tiles (pipeline rounds)
NSLAB = T // SLAB         # 4 slabs per batch
ONES_LAG = 2              # pairs of lag before the ones-matmul reads exp

_NC_CACHE = None


def build(krep=1, bench_internal_a=False, no_adma=False, phase1_only=False,
          no_rmm=False, bench_no_io=False, r_from_e=False):
    """krep>1 wraps the whole body in a hardware loop that recomputes the
    same outputs krep times — used only for differential HW timing.
    bench_internal_a keeps A in device DRAM (not transferred back).
    bench_no_io keeps ALL tensors on device (zero-filled once) with a dummy
    4-byte output, so call wall time is dispatch + execute only.
    no_adma / phase1_only / no_rmm are timing-bisection modes (wrong
    outputs)."""
    nc = bacc.Bacc(None, target_bir_lowering=False)

    io_kind = "Internal" if bench_no_io else "ExternalInput"
    K_d = nc.dram_tensor("K", [NB, CX, T], f32r, kind=io_kind)
    V_d = nc.dram_tensor("V", [NB, CX, T], f32r, kind=io_kind)
    Q_d = nc.dram_tensor("Q", [NB, CX, T], f32r, kind=io_kind)
    a_kind = ("Internal" if (bench_internal_a or bench_no_io)
              else "ExternalOutput")
    A_d = nc.dram_tensor("A", [NB, T, T], f32r, kind=a_kind)
    r_kind = "Internal" if bench_no_io else "ExternalOutput"
    R_d = nc.dram_tensor("R", [NB, CX, T], f32, kind=r_kind)
    dummy_d = (nc.dram_tensor("bench_out", [1, 1], f32, kind="ExternalOutput")
               if bench_no_io else None)

    with tile.TileContext(nc) as tc:
        with (
            tc.tile_pool(name="consts", bufs=1) as consts,
            tc.tile_pool(name="kq", bufs=2) as kq,
            tc.tile_pool(name="vv", bufs=1) as vv,
            tc.tile_pool(name="vt", bufs=2) as vtp,
            tc.tile_pool(name="en", bufs=12) as enp,
            tc.tile_pool(name="aj", bufs=4) as ajp,
            tc.tile_pool(name="rout", bufs=4) as routp,
            tc.tile_pool(name="rmisc", bufs=3) as rmisc,
            tc.tile_pool(name="ps", bufs=2, space="PSUM") as psp,
            tc.tile_pool(name="ps_ones", bufs=2, space="PSUM") as ps_ones,
            tc.tile_pool(name="ps_r", bufs=2, space="PSUM") as ps_r,
        ):
            ident = consts.tile([P, P], f32)
            make_identity(nc, ident)
            identr = consts.tile([P, P], f32r)
            nc.vector.tensor_copy(identr[:], ident[:])
            ones_f = consts.tile([P, 1], f32)
            nc.vector.memset(ones_f[:], 1.0)
            ones = consts.tile([P, 1], f32r)
            nc.vector.tensor_copy(ones[:], ones_f[:])

            jobs = [(b, s) for b in range(NB) for s in range(NSLAB)]
            kq_tiles = {}     # b -> (k_sb, q_sb)
            vt_tiles = {}     # b -> vt
            jstate = {}       # job index -> (en_pairs, ones_psum, pending)
            v_pending = {}    # b -> v_sb awaiting VT build

            def emit_kqv_load(b):
                k_sb = kq.tile([P, CXO, T], f32r, tag="K", name=f"k{b}")
                q_sb = kq.tile([P, CXO, T], f32r, tag="Q", name=f"q{b}")
                kq_tiles[b] = (k_sb, q_sb)
                kre = K_d[b].rearrange("(o p) t -> p o t", p=P)
                qre = Q_d[b].rearrange("(o p) t -> p o t", p=P)
                # slab-0 phase A first needs q[0:512] and k progressively
                cuts = [(q_sb, qre, 0, 512), (k_sb, kre, 0, 512),
                        (k_sb, kre, 512, 1024), (k_sb, kre, 1024, 1536),
                        (k_sb, kre, 1536, 2048), (q_sb, qre, 512, 1024),
                        (q_sb, qre, 1024, 1536), (q_sb, qre, 1536, 2048)]
                for dst, src, lo, hi in cuts:
                    nc.sync.dma_start(dst[:, :, lo:hi], src[:, :, lo:hi])
                v_sb = vv.tile([P, CXO, T], f32r, tag="V", name=f"v{b}")
                nc.sync.dma_start(
                    v_sb[:], V_d[b].rearrange("(o p) t -> p o t", p=P))
                return v_sb

            def emit_vt_build(b, v_sb):
                vt = vtp.tile([P, NJ, CX], f32r, tag="VT", name=f"vt{b}")
                vt_tiles[b] = vt
                for j in range(NJ):
                    for c in range(CXO):
                        pvt = psp.tile([P, P], f32r, tag="work", name="pvt")
                        nc.tensor.transpose(
                            pvt[:], v_sb[:, c, j * P:(j + 1) * P], identr[:])
                        nc.vector.tensor_copy(
                            vt[:, j, c * P:(c + 1) * P], pvt[:])

            def _get_state(ji):
                if ji not in jstate:
                    jstate[ji] = ([None] * NJ2,
                                  ps_ones.tile([1, SLAB], f32, tag="ones",
                                               name=f"os{ji}"),
                                  [])
                return jstate[ji]

            def emit_ones_mm(ji, jp):
                en_pairs, osum, _ = jstate[ji]
                en = en_pairs[jp]
                for h in range(2):
                    nc.tensor.matmul(
                        osum[:], ones[:], en[:, h, :],
                        start=(jp == 0 and h == 0),
                        stop=(jp == NJ2 - 1 and h == 1))

            def emit_phaseA_pair(ji, jp):
                """Scores matmuls + exp for one pair of tx tiles; queues the
                ones-matmul to run ONES_LAG pairs later (so the in-order PE
                stream doesn't wait on ScalarE's exp)."""
                b, s = jobs[ji]
                en_pairs, osum, pending = _get_state(ji)
                k_sb, q_sb = kq_tiles[b]
                ty0 = s * SLAB
                en = enp.tile([P, 2, SLAB], f32r, tag="en",
                              name=f"en{ji}_{jp}")
                en_pairs[jp] = en
                pss = psp.tile([P, 2, SLAB], f32, tag="work", name="pss")
                for h in range(2):
                    j = jp * 2 + h
                    for c in range(CXO):
                        nc.tensor.matmul(
                            pss[:, h, :],
                            k_sb[:, c, j * P:(j + 1) * P],
                            q_sb[:, c, ty0:ty0 + SLAB],
                            start=(c == 0), stop=(c == CXO - 1))
                nc.scalar.activation(
                    out=en[:], in_=pss[:],
                    func=mybir.ActivationFunctionType.Exp, scale=SCALE)
                pending.append(jp)
                if len(pending) > ONES_LAG:
                    emit_ones_mm(ji, pending.pop(0))
                if jp == NJ2 - 1:
                    while pending:
                        emit_ones_mm(ji, pending.pop(0))

            def emit_rpath(ji):
                """r = 1/colsums on the [1, SLAB] psum row, broadcast to all
                partitions. No PE involvement."""
                _, osum, _ = jstate[ji]
                rrow = rmisc.tile([1, SLAB], f32, tag="rrow")
                nc.vector.reciprocal(rrow[:], osum[:])
                rbc = rmisc.tile([P, SLAB], f32, tag="rbc")
                nc.gpsimd.partition_broadcast(rbc[:], rrow[:])
                return rbc

            def emit_phaseB_pair(ji, jp, rbc, pr):
                """R accumulation straight from the unnormalized exp tiles
                (R = (V.E).diag(r), normalized at psum evacuation), plus the
                A-output normalize-copy as a VectorE/DMA side stream."""
                b, s = jobs[ji]
                en_pairs, _, _ = jstate[ji]
                vt = vt_tiles[b]
                ty0 = s * SLAB
                en = en_pairs[jp]
                if not no_rmm and r_from_e:
                    for h in range(2):
                        j = jp * 2 + h
                        for c in range(CXO):
                            nc.tensor.matmul(
                                pr[c][:], vt[:, j, c * P:(c + 1) * P],
                                en[:, h, :],
                                start=(j == 0), stop=(j == NJ - 1))
                aj = ajp.tile([P, 2, SLAB], f32r, tag="aj")
                nc.vector.tensor_tensor(
                    aj[:], en[:],
                    rbc[:].unsqueeze(1).to_broadcast((P, 2, SLAB)),
                    mybir.AluOpType.mult)
                for h in range(2):
                    j = jp * 2 + h
                    if not no_adma:
                        nc.sync.dma_start(
                            A_d[b, j * P:(j + 1) * P, ty0:ty0 + SLAB],
                            aj[:, h, :])
                    if not no_rmm and not r_from_e:
                        for c in range(CXO):
                            nc.tensor.matmul(
                                pr[c][:], vt[:, j, c * P:(c + 1) * P],
                                aj[:, h, :],
                                start=(j == 0), stop=(j == NJ - 1))

            def next_job_emitters(nji):
                """Emitters for job nji's phase A (+ VT build at batch start).
                K/Q/V loads are prefetched separately two jobs ahead."""
                if nji >= len(jobs):
                    return []
                b, s = jobs[nji]
                ems = []
                if b not in kq_tiles:
                    ems.append(lambda b=b: v_pending.__setitem__(
                        b, emit_kqv_load(b)))
                for jp in range(NJ2):
                    ems.append(lambda ji=nji, jp=jp: emit_phaseA_pair(ji, jp))
                if s == 0:
                    ems.append(lambda b=b: emit_vt_build(b, v_pending.pop(b)))
                return ems

            # ---- main pipeline ----
            def _pipeline():
              kq_tiles.clear()
              vt_tiles.clear()
              v_pending.clear()
              jstate.clear()
              for e in next_job_emitters(0):
                e()
              for ji, (b, s) in enumerate(jobs):
                rbc = emit_rpath(ji)
                # prefetch next batch's inputs two jobs ahead
                if ji + 2 < len(jobs):
                    nb = jobs[ji + 2][0]
                    if nb not in kq_tiles:
                        v_pending[nb] = emit_kqv_load(nb)
                if phase1_only:
                    for e in next_job_emitters(ji + 1):
                        e()
                    for c in range(CXO):
                        r_sb = routp.tile([P, SLAB], f32, tag="rsb")
                        nc.vector.memset(r_sb[:], 0.0)
                        nc.sync.dma_start(
                            R_d[b, c * P:(c + 1) * P,
                                s * SLAB:(s + 1) * SLAB], r_sb[:])
                    del jstate[ji]
                    continue
                nxt = next_job_emitters(ji + 1)
                pr = [ps_r.tile([P, SLAB], f32, tag="psr", name=f"psr{ji}_{c}")
                      for c in range(CXO)]
                # lead with next-job phase-A work so PE has rbc-independent
                # instructions while the r-path resolves
                for _ in range(2):
                    if nxt:
                        nxt.pop(0)()
                for jp in range(NJ2):
                    emit_phaseB_pair(ji, jp, rbc, pr)
                    if nxt:
                        nxt.pop(0)()
                while nxt:
                    nxt.pop(0)()
                for c in range(CXO):
                    r_sb = routp.tile([P, SLAB], f32, tag="rsb")
                    if no_rmm:
                        nc.vector.memset(r_sb[:], 0.0)
                    elif r_from_e:
                        # normalize R during psum evacuation: R = (V.E) * r
                        nc.vector.tensor_tensor(
                            r_sb[:], pr[c][:], rbc[:], mybir.AluOpType.mult)
                    else:
                        nc.vector.tensor_copy(r_sb[:], pr[c][:])
                    nc.sync.dma_start(
                        R_d[b, c * P:(c + 1) * P, s * SLAB:(s + 1) * SLAB],
                        r_sb[:])
                del jstate[ji]

            if bench_no_io:
                # zero-fill the internal input tensors once so the compute
                # sees finite data (exp(0)=1), and write the dummy output
                zsb = consts.tile([P, 2048], f32r, name="zsb")
                nc.vector.memset(zsb[:].bitcast(f32), 0.0)
                for td in (K_d, V_d, Q_d):
                    flat = td[:].rearrange("b c t -> (b c) t")
                    for h in range(4):
                        nc.sync.dma_start(flat[h * P:(h + 1) * P, :], zsb[:])
                dsb = consts.tile([1, 1], f32, name="dsb")
                nc.vector.memset(dsb[:], 0.0)
                nc.sync.dma_start(dummy_d[:], dsb[:])

            if krep > 1:
                with tc.For_i(0, krep, 1):
                    _pipeline()
            else:
                _pipeline()

    nc.compile()
    return nc


def kernel(K, V, Q):
    global _NC_CACHE
    K = np.ascontiguousarray(np.asarray(K, dtype=np.float32))
    V = np.ascontiguousarray(np.asarray(V, dtype=np.float32))
    Q = np.ascontiguousarray(np.asarray(Q, dtype=np.float32))
    N = K.shape[0]
    assert K.shape == (N, CX, T) and V.shape == (N, CX, T) and Q.shape == (N, CX, T)

    if _NC_CACHE is None:
        _NC_CACHE = build()
    nc = _NC_CACHE

    in_maps = [
        {"K": K[c * NB:(c + 1) * NB], "V": V[c * NB:(c + 1) * NB],
         "Q": Q[c * NB:(c + 1) * NB]}
        for c in range(N_CORES)
    ]
    res = None
    for attempt in range(3):
        try:
            res = run_bass_kernel_spmd(nc, in_maps,
                                       core_ids=list(range(N_CORES)))
            break
        except Exception:
            # transient NRT_EXEC_UNIT_UNRECOVERABLE right after another
            # process released the cores — back off and retry
            if attempt == 2:
                raise
            import time as _time
            _time.sleep(20)
            try:
                import jax
                jax.clear_backends()
            except Exception:
                pass
    A = np.concatenate([r["A"] for r in res.results], axis=0)
    R = np.concatenate([r["R"] for r in res.results], axis=0)
    return (R, A)
